# revision 43
# baseline (speedup 1.0000x reference)
"""Trainium2 Bass kernel for nn_AttentionFlow (trilinear attention flow layer).

Full inputs -> shard batch over 8 NeuronCores (2 batches/core) -> gather.

Per batch (C [1024,768], Q [128,768]):
  S[i,j] = w1.C_i + w2.Q_j + (C_i*w3).Q_j   (c_logit dropped from columns:
           softmax over j is invariant to per-row constants)
  C2Q = softmax_j(masked S); A = C2Q @ Q
  Q2C = softmax_i(c-masked rowmax of raw S); Bctx = Q2C @ C
  out = [C | A | C*A | C*Bctx]

v2 layout strategy (vs the all-fp32 v1):
  - The Q-side prep (Q^T*w3 with the appended w1 column, q_logit row, q-mask
    row) is precomputed on the host; nothing q-side is transposed on device.
  - Scores stay fp32 on the PE: the Q2C path takes exp of per-row maxima, so
    ~0.1-abs bf16 score noise turns into >5% weight flips between competing
    context rows and blows the C*B tolerance (measured: 1.3 abs err).
  - The A matmul runs bf16 (E^T copied to bf16, host-cast bf16 Q): softmax
    weights tolerate 0.4% relative error fine (measured 4e-3 end-to-end).
  - Bctx runs bf16 (f32r needs explicitly pre-rounded producers per the BIR
    verifier, which costs the same extra copy); C tiles are converted to bf16
    on the ACT engine during the transpose lookahead stage.
  - Output blocks stream independently: [A | C*A] from one staging tile,
    C*B later; all on the SP HWDGE ring while loads use the ACT ring.
  - The C passthrough block (out[:, :, 0:D] = C) is a bit-exact copy of an
    input, so it is filled during the host-side gather/concat step instead of
    being round-tripped through the device store path.
"""

from contextlib import ExitStack

import numpy as np
import ml_dtypes

import concourse.bass as bass
import concourse.tile as tile
from concourse import bacc, mybir
from concourse.bass_utils import run_bass_kernel_spmd
from concourse.masks import make_identity

F32 = mybir.dt.float32
BF16 = mybir.dt.bfloat16
AX = mybir.AluOpType
ACTF = mybir.ActivationFunctionType

NEG = np.float32(-1e9)
NCORES = 8
NB = 2           # batches per core
N = 1024         # context length
M = 128          # query length
D = 768          # feature dim
NT = N // 128    # n-tiles per batch
KC = D // 128    # contraction chunks

_CACHE: dict = {}


def _build_program(iters: int = 1) -> bass.Bass:
    nc = bacc.Bacc("TRN2", target_bir_lowering=False, debug=False)
    C_d = nc.declare_dram_parameter("C", [NB, N, D], F32, isOutput=False)
    Ct_d = nc.declare_dram_parameter("Ct", [NB, KC, 128, N], F32,
                                     isOutput=False)
    qa_d = nc.declare_dram_parameter("qa", [NB, 128, KC, M + 1], F32,
                                     isOutput=False)
    qrow_d = nc.declare_dram_parameter("qrow", [NB, 1, M + 1], F32,
                                       isOutput=False)
    qmB_d = nc.declare_dram_parameter("qmB", [NB, 128, M], F32,
                                      isOutput=False)
    Qb_d = nc.declare_dram_parameter("Qb", [NB, M, D], BF16, isOutput=False)
    cmT_d = nc.declare_dram_parameter("cmT", [NB, 128, NT], F32, isOutput=False)
    out_d = nc.declare_dram_parameter("out", [NB, N, 4 * D], F32, isOutput=True)

    with ExitStack() as ctx:
        tc = ctx.enter_context(tile.TileContext(nc))
        consts = ctx.enter_context(tc.tile_pool(name="consts", bufs=1))
        cpool = ctx.enter_context(tc.tile_pool(name="cpool", bufs=2))
        cbpool = ctx.enter_context(tc.tile_pool(name="cbpool", bufs=2))
        ctpool = ctx.enter_context(tc.tile_pool(name="ctpool", bufs=2))
        qpool = ctx.enter_context(tc.tile_pool(name="qpool", bufs=2))
        epool = ctx.enter_context(tc.tile_pool(name="epool", bufs=6))
        etpool = ctx.enter_context(tc.tile_pool(name="etpool", bufs=6))
        spool = ctx.enter_context(tc.tile_pool(name="spool", bufs=4))
        stA = ctx.enter_context(tc.tile_pool(name="stA", bufs=4))
        stB = ctx.enter_context(tc.tile_pool(name="stB", bufs=3))
        ps_t = ctx.enter_context(tc.tile_pool(name="ps_t", bufs=1, space="PSUM"))
        ps_s = ctx.enter_context(tc.tile_pool(name="ps_s", bufs=3, space="PSUM"))
        ps_a = ctx.enter_context(tc.tile_pool(name="ps_a", bufs=2, space="PSUM"))

        ident = consts.tile([128, 128], F32)
        make_identity(nc, ident)
        ones_row = consts.tile([1, 128], F32)
        nc.vector.memset(ones_row, 1.0)
        ones_col = consts.tile([128, 1], F32)
        nc.vector.memset(ones_col, 1.0)
        ones_row_bf = consts.tile([1, 128], BF16)
        nc.vector.memset(ones_row_bf, 1.0)

        loop_ctx = tc.For_i(0, iters, 1) if iters > 1 else None
        if loop_ctx is not None:
            ctx.enter_context(loop_ctx)
        for b in range(NB):
            # ------- loads (ACT HWDGE ring; stores use SP HWDGE ring) -------
            c_big = cpool.tile([128, NT, D], F32, tag="c")
            for t in range(NT):
                nc.scalar.dma_start(out=c_big[:, t, :],
                                    in_=C_d[b, t * 128:(t + 1) * 128, :])
            # host-pre-transposed C^T: kills 48 PE transposes + psum copies
            ct = ctpool.tile([128, KC, N], F32, tag="ct")
            nc.scalar.dma_start(
                out=ct, in_=Ct_d[b].rearrange("c p n -> p c n"))
            qa_t = qpool.tile([128, KC, M + 1], F32, tag="qa")
            nc.scalar.dma_start(out=qa_t, in_=qa_d[b])
            qb_t = qpool.tile([M, D], BF16, tag="qb")
            nc.scalar.dma_start(out=qb_t, in_=Qb_d[b])
            qrow = qpool.tile([1, M + 1], F32, tag="qrow")
            nc.scalar.dma_start(out=qrow, in_=qrow_d[b])
            qmB = qpool.tile([128, M], F32, tag="qmB")
            nc.scalar.dma_start(out=qmB, in_=qmB_d[b])
            cmT = spool.tile([128, NT], F32, tag="cmT")
            nc.scalar.dma_start(out=cmT, in_=cmT_d[b])

            # mask derivations: s0=1-cm, cmN=-1e9*cm
            s0c = spool.tile([128, NT], F32, tag="s0c")
            nc.vector.tensor_scalar(out=s0c, in0=cmT, scalar1=-1.0, scalar2=1.0,
                                    op0=AX.mult, op1=AX.add)
            cmNc = spool.tile([128, NT], F32, tag="cmNc")
            nc.vector.tensor_scalar_mul(out=cmNc, in0=cmT, scalar1=float(NEG))

            cb = cbpool.tile([128, NT, D], BF16, tag="cb")
            G = spool.tile([128, NT], F32, tag="G")
            for t in range(NT):
                nc.gpsimd.tensor_copy(out=cb[:, t, :], in_=c_big[:, t, :])
                s_ps = ps_s.tile([128, M + 1], F32, tag="sps")
                for c in range(KC):
                    nc.tensor.matmul(s_ps,
                                     lhsT=ct[:, c, t * 128:(t + 1) * 128],
                                     rhs=qa_t[:, c, :], start=(c == 0),
                                     stop=False)
                nc.tensor.matmul(s_ps, lhsT=ones_row, rhs=qrow, start=False,
                                 stop=True)
                rawmax = spool.tile([128, 1], F32, tag="rawmax")
                nc.vector.reduce_max(out=rawmax, in_=s_ps[:, 0:M],
                                     axis=mybir.AxisListType.X)
                # q2c column: (rawmax + c_logit)*s0 - 1e9*cm
                rawc = spool.tile([128, 1], F32, tag="rawc")
                nc.vector.tensor_add(out=rawc, in0=rawmax, in1=s_ps[:, M:M + 1])
                nc.vector.tensor_scalar(out=G[:, t:t + 1], in0=rawc,
                                        scalar1=s0c[:, t:t + 1],
                                        scalar2=cmNc[:, t:t + 1],
                                        op0=AX.mult, op1=AX.add)
                # q-mask add lands in SBUF (qmB is the host-prebroadcast mask
                # row): frees s_ps after 3 DVE reads, PE never re-opens it
                Sm = spool.tile([128, M], F32, tag="Sm")
                nc.vector.tensor_add(out=Sm, in0=s_ps[:, 0:M], in1=qmB)
                nshmax = spool.tile([128, 1], F32, tag="nshmax")
                nc.vector.reduce_max(out=nshmax, in_=Sm,
                                     axis=mybir.AxisListType.X, negate=True)
                biasT = spool.tile([128, 1], F32, tag="biasT")
                nc.vector.tensor_scalar_mul(out=biasT, in0=nshmax,
                                            scalar1=s0c[:, t:t + 1])
                E = epool.tile([128, M], F32, tag="E")
                Zrow = spool.tile([128, 1], F32, tag="Zrow")
                nc.scalar.activation(out=E, in_=Sm, func=ACTF.Exp,
                                     bias=biasT, scale=s0c[:, t:t + 1],
                                     accum_out=Zrow)
                zr = spool.tile([128, 1], F32, tag="zr")
                nc.vector.reciprocal(out=zr, in_=Zrow)

                # A = (E @ Q) * zr in bf16; C*A fused off the staged A
                et_ps = ps_t.tile([128, M], F32, tag="pst")
                nc.tensor.transpose(et_ps, E, ident)
                et = etpool.tile([128, M], BF16, tag="et")
                nc.vector.tensor_copy(out=et, in_=et_ps)
                a_ps = ps_a.tile([128, D], F32, tag="aps")
                nc.tensor.matmul(a_ps[:, 0:512], lhsT=et, rhs=qb_t[:, 0:512],
                                 start=True, stop=True)
                nc.tensor.matmul(a_ps[:, 512:D], lhsT=et, rhs=qb_t[:, 512:D],
                                 start=True, stop=True)
                stage = stA.tile([128, 2 * D], F32, tag="stA")
                nc.scalar.activation(out=stage[:, 0:D], in_=a_ps,
                                     func=ACTF.Copy, scale=zr)
                ca_eng = nc.gpsimd if t % 2 == 0 else nc.vector
                ca_eng.tensor_mul(out=stage[:, D:2 * D],
                                  in0=stage[:, 0:D], in1=c_big[:, t, :])
                nc.sync.dma_start(
                    out=out_d[b, t * 128:(t + 1) * 128, D:3 * D], in_=stage)

            # ---------------- Q2C global softmax + Bctx ----------------
            # G is already in column space [128, NT]; global max via one tiny
            # transpose, then exp/sum stay in column space (no row-space trip).
            nmax = spool.tile([128, 1], F32, tag="nmax")
            nc.vector.reduce_max(out=nmax, in_=G, axis=mybir.AxisListType.X,
                                 negate=True)
            t1_ps = ps_t.tile([1, 128], F32, tag="pst")
            nc.tensor.transpose(t1_ps, nmax, ident)
            negMg = spool.tile([1, 1], F32, tag="negMg")
            nc.vector.tensor_reduce(out=negMg, in_=t1_ps,
                                    axis=mybir.AxisListType.X, op=AX.min)
            nmb_ps = ps_t.tile([128, 1], F32, tag="pst")
            nc.tensor.matmul(nmb_ps, lhsT=ones_row, rhs=negMg, start=True,
                             stop=True)
            nmb = spool.tile([128, 1], F32, tag="nmb")
            nc.vector.tensor_copy(out=nmb, in_=nmb_ps)
            er = spool.tile([128, NT], F32, tag="er")
            zq = spool.tile([128, 1], F32, tag="zq")
            nc.scalar.activation(out=er, in_=G, func=ACTF.Exp, bias=nmb,
                                 accum_out=zq)
            zs_ps = ps_t.tile([1, 1], F32, tag="pst")
            nc.tensor.matmul(zs_ps, lhsT=zq, rhs=ones_col, start=True,
                             stop=True)
            zqr = spool.tile([1, 1], F32, tag="zqr")
            nc.vector.reciprocal(out=zqr, in_=zs_ps)
            # Bctx = (er/Z) @ C in bf16 (1 cycle/row)
            er_bf = spool.tile([128, NT], BF16, tag="erbf")
            nc.vector.tensor_copy(out=er_bf, in_=er)
            bctx_ps = ps_a.tile([1, D], F32, tag="aps")
            for t in range(NT):
                nc.tensor.matmul(bctx_ps[:, 0:512],
                                 lhsT=er_bf[:, t:t + 1],
                                 rhs=cb[:, t, 0:512],
                                 start=(t == 0), stop=(t == NT - 1))
                nc.tensor.matmul(bctx_ps[:, 512:D],
                                 lhsT=er_bf[:, t:t + 1],
                                 rhs=cb[:, t, 512:D],
                                 start=(t == 0), stop=(t == NT - 1))
            bctx_bf = spool.tile([1, D], BF16, tag="bctxb")
            nc.scalar.activation(out=bctx_bf, in_=bctx_ps, func=ACTF.Copy,
                                 scale=zqr)
            # broadcast bctx row to 128 partitions (PE), then multiply per tile
            bb_ps = ps_a.tile([128, D], F32, tag="aps")
            nc.tensor.matmul(bb_ps[:, 0:512], lhsT=ones_row_bf,
                             rhs=bctx_bf[:, 0:512], start=True, stop=True)
            nc.tensor.matmul(bb_ps[:, 512:D], lhsT=ones_row_bf,
                             rhs=bctx_bf[:, 512:D], start=True, stop=True)
            Bb = qpool.tile([128, D], F32, tag="Bb")
            nc.scalar.copy(out=Bb, in_=bb_ps)
            for t in range(NT):
                sb = stB.tile([128, D], F32, tag="stB")
                cb_eng = nc.gpsimd if t % 2 == 1 else nc.vector
                cb_eng.tensor_mul(out=sb, in0=c_big[:, t, :], in1=Bb)
                # alternate C*B stores across the two HWDGE rings
                st_eng = nc.scalar if t % 2 == 0 else nc.sync
                st_eng.dma_start(
                    out=out_d[b, t * 128:(t + 1) * 128, 3 * D:4 * D], in_=sb)
    nc.compile()
    return nc


def _get_program() -> bass.Bass:
    if "nc" not in _CACHE:
        _CACHE["nc"] = _build_program()
    return _CACHE["nc"]


def _make_in_maps(inputs) -> list:
    C = np.ascontiguousarray(np.asarray(inputs["C"], dtype=np.float32))
    Q = np.ascontiguousarray(np.asarray(inputs["Q"], dtype=np.float32))
    c_mask = np.asarray(inputs["c_mask"])
    q_mask = np.asarray(inputs["q_mask"])
    w1 = np.asarray(inputs["w1"], dtype=np.float32).reshape(-1)
    w2 = np.asarray(inputs["w2"], dtype=np.float32).reshape(-1)
    w3 = np.asarray(inputs["w3"], dtype=np.float32).reshape(-1)
    B = C.shape[0]

    # Ct[b, c, p, n] = C[b, n, c*128+p] (pre-transposed score operand)
    Ct = np.ascontiguousarray(
        C.transpose(0, 2, 1).reshape(B, KC, 128, N))

    # qa[b, p, c, j] = Q[b, j, c*128+p] * w3[c*128+p]; col M = w1[c*128+p]
    qa = np.empty((B, 128, KC, M + 1), np.float32)
    Qt = Q.transpose(0, 2, 1).reshape(B, KC, 128, M)          # [B, c, p, j]
    qa[:, :, :, 0:M] = (Qt * w3.reshape(1, KC, 128, 1)).transpose(0, 2, 1, 3)
    qa[:, :, :, M] = w1.reshape(KC, 128).T[None, :, :]

    qrow = np.zeros((B, 1, M + 1), np.float32)
    qrow[:, 0, 0:M] = Q @ w2
    # q-mask pre-broadcast to all 128 partitions: device adds it with one
    # DVE op instead of a rank-1 matmul back into the score psum
    qmB = np.ascontiguousarray(np.broadcast_to(
        q_mask[:, 0:1, :].astype(np.float32) * NEG, (B, 128, M)))
    Qb = Q.astype(ml_dtypes.bfloat16)
    # cmT[b, p, t] = c_mask[b, 0, t*128 + p]
    cmT = np.ascontiguousarray(
        c_mask[:, 0, :].astype(np.float32).reshape(B, NT, 128).transpose(0, 2, 1))

    in_maps = []
    for core in range(NCORES):
        sl = slice(core * NB, (core + 1) * NB)
        in_maps.append({
            "C": C[sl],
            "Ct": np.ascontiguousarray(Ct[sl]),
            "qa": np.ascontiguousarray(qa[sl]),
            "qrow": np.ascontiguousarray(qrow[sl]),
            "qmB": np.ascontiguousarray(qmB[sl]),
            "Qb": np.ascontiguousarray(Qb[sl]),
            "cmT": np.ascontiguousarray(cmT[sl]),
        })
    return in_maps


def kernel(**inputs) -> np.ndarray:
    nc = _get_program()
    in_maps = _make_in_maps(inputs)
    res = run_bass_kernel_spmd(nc, in_maps, list(range(NCORES)))
    out = np.concatenate([r["out"] for r in res.results], axis=0)
    # C passthrough block, filled during the gather (bit-exact input copy)
    out[:, :, 0:D] = np.asarray(inputs["C"], dtype=np.float32)
    return out


# revision 45
# speedup vs baseline: 1.1111x; 1.1111x over previous
"""Trainium2 Bass kernel for nn_AttentionFlow (trilinear attention flow layer).

Full inputs -> shard batch over 8 NeuronCores (2 batches/core) -> gather.

Per batch (C [1024,768], Q [128,768]):
  S[i,j] = w1.C_i + w2.Q_j + (C_i*w3).Q_j   (c_logit dropped from columns:
           softmax over j is invariant to per-row constants)
  C2Q = softmax_j(masked S); A = C2Q @ Q
  Q2C = softmax_i(c-masked rowmax of raw S); Bctx = Q2C @ C
  out = [C | A | C*A | C*Bctx]

v2 layout strategy (vs the all-fp32 v1):
  - The Q-side prep (Q^T*w3 with the appended w1 column, q_logit row, q-mask
    row) is precomputed on the host; nothing q-side is transposed on device.
  - Scores stay fp32 on the PE: the Q2C path takes exp of per-row maxima, so
    ~0.1-abs bf16 score noise turns into >5% weight flips between competing
    context rows and blows the C*B tolerance (measured: 1.3 abs err).
  - The A matmul runs bf16 (E^T copied to bf16, host-cast bf16 Q): softmax
    weights tolerate 0.4% relative error fine (measured 4e-3 end-to-end).
  - Bctx runs bf16 (f32r needs explicitly pre-rounded producers per the BIR
    verifier, which costs the same extra copy); C tiles are converted to bf16
    on the ACT engine during the transpose lookahead stage.
  - Output blocks stream independently: [A | C*A] from one staging tile,
    C*B later; all on the SP HWDGE ring while loads use the ACT ring.
  - The C passthrough block (out[:, :, 0:D] = C) is a bit-exact copy of an
    input, so it is filled during the host-side gather/concat step instead of
    being round-tripped through the device store path.
"""

from contextlib import ExitStack

import numpy as np
import ml_dtypes

import concourse.bass as bass
import concourse.tile as tile
from concourse import bacc, mybir
from concourse.bass_utils import run_bass_kernel_spmd
from concourse.masks import make_identity

F32 = mybir.dt.float32
BF16 = mybir.dt.bfloat16
AX = mybir.AluOpType
ACTF = mybir.ActivationFunctionType

NEG = np.float32(-1e9)
NCORES = 8
NB = 2           # batches per core
N = 1024         # context length
M = 128          # query length
D = 768          # feature dim
NT = N // 128    # n-tiles per batch
KC = D // 128    # contraction chunks

_CACHE: dict = {}


def _build_program(iters: int = 1) -> bass.Bass:
    nc = bacc.Bacc("TRN2", target_bir_lowering=False, debug=False)
    C_d = nc.declare_dram_parameter("C", [NB, N, D], F32, isOutput=False)
    Ct_d = nc.declare_dram_parameter("Ct", [NB, KC, 128, N], F32,
                                     isOutput=False)
    qa_d = nc.declare_dram_parameter("qa", [NB, 128, KC, M + 1], F32,
                                     isOutput=False)
    qrow_d = nc.declare_dram_parameter("qrow", [NB, 1, M + 1], F32,
                                       isOutput=False)
    qmB_d = nc.declare_dram_parameter("qmB", [NB, 128, M], F32,
                                      isOutput=False)
    Qb_d = nc.declare_dram_parameter("Qb", [NB, M, D], BF16, isOutput=False)
    cmT_d = nc.declare_dram_parameter("cmT", [NB, 128, NT], F32, isOutput=False)
    out_d = nc.declare_dram_parameter("out", [NB, N, 4 * D], F32, isOutput=True)

    with ExitStack() as ctx:
        tc = ctx.enter_context(tile.TileContext(nc))
        consts = ctx.enter_context(tc.tile_pool(name="consts", bufs=1))
        cpool = ctx.enter_context(tc.tile_pool(name="cpool", bufs=2))
        cbpool = ctx.enter_context(tc.tile_pool(name="cbpool", bufs=2))
        ctpool = ctx.enter_context(tc.tile_pool(name="ctpool", bufs=2))
        qpool = ctx.enter_context(tc.tile_pool(name="qpool", bufs=2))
        epool = ctx.enter_context(tc.tile_pool(name="epool", bufs=6))
        etpool = ctx.enter_context(tc.tile_pool(name="etpool", bufs=6))
        spool = ctx.enter_context(tc.tile_pool(name="spool", bufs=4))
        stA = ctx.enter_context(tc.tile_pool(name="stA", bufs=4))
        stB = ctx.enter_context(tc.tile_pool(name="stB", bufs=3))
        ps_t = ctx.enter_context(tc.tile_pool(name="ps_t", bufs=1, space="PSUM"))
        ps_s = ctx.enter_context(tc.tile_pool(name="ps_s", bufs=3, space="PSUM"))
        ps_a = ctx.enter_context(tc.tile_pool(name="ps_a", bufs=2, space="PSUM"))

        ident = consts.tile([128, 128], F32)
        make_identity(nc, ident)
        ones_row = consts.tile([1, 128], F32)
        nc.vector.memset(ones_row, 1.0)
        ones_col = consts.tile([128, 1], F32)
        nc.vector.memset(ones_col, 1.0)
        ones_row_bf = consts.tile([1, 128], BF16)
        nc.vector.memset(ones_row_bf, 1.0)

        loop_ctx = tc.For_i(0, iters, 1) if iters > 1 else None
        if loop_ctx is not None:
            ctx.enter_context(loop_ctx)
        for b in range(NB):
            # ------- loads (ACT HWDGE ring; stores use SP HWDGE ring) -------
            c_big = cpool.tile([128, NT, D], F32, tag="c")
            for t in range(NT):
                nc.scalar.dma_start(out=c_big[:, t, :],
                                    in_=C_d[b, t * 128:(t + 1) * 128, :])
            # host-pre-transposed C^T: kills 48 PE transposes + psum copies
            ct = ctpool.tile([128, KC, N], F32, tag="ct")
            nc.scalar.dma_start(
                out=ct, in_=Ct_d[b].rearrange("c p n -> p c n"))
            qa_t = qpool.tile([128, KC, M + 1], F32, tag="qa")
            nc.scalar.dma_start(out=qa_t, in_=qa_d[b])
            qb_t = qpool.tile([M, D], BF16, tag="qb")
            nc.scalar.dma_start(out=qb_t, in_=Qb_d[b])
            qrow = qpool.tile([1, M + 1], F32, tag="qrow")
            nc.scalar.dma_start(out=qrow, in_=qrow_d[b])
            qmB = qpool.tile([128, M], F32, tag="qmB")
            nc.scalar.dma_start(out=qmB, in_=qmB_d[b])
            cmT = spool.tile([128, NT], F32, tag="cmT")
            nc.scalar.dma_start(out=cmT, in_=cmT_d[b])

            # mask derivations: s0=1-cm, cmN=-1e9*cm
            s0c = spool.tile([128, NT], F32, tag="s0c")
            nc.vector.tensor_scalar(out=s0c, in0=cmT, scalar1=-1.0, scalar2=1.0,
                                    op0=AX.mult, op1=AX.add)
            cmNc = spool.tile([128, NT], F32, tag="cmNc")
            nc.vector.tensor_scalar_mul(out=cmNc, in0=cmT, scalar1=float(NEG))

            cb = cbpool.tile([128, NT, D], BF16, tag="cb")
            # per-tile row maxima / c_logits accumulate into columns; the G
            # combine runs once per batch (3 wide DVE ops vs 16 tiny ones)
            RM = spool.tile([128, NT], F32, tag="RM")
            CL = spool.tile([128, NT], F32, tag="CL")
            # software-pipelined: the E^T/A stage for tile t issues AFTER the
            # score matmuls of tile t+1, so the in-order PE stream never waits
            # on the exp chain; DVE issues critical ops (exp path) first
            Es, Zs = [None] * NT, [None] * NT
            for step in range(NT + 1):
                s = step
                if s < NT:
                    nc.gpsimd.tensor_copy(out=cb[:, s, :], in_=c_big[:, s, :])
                    s_ps = ps_s.tile([128, M + 1], F32, tag="sps")
                    for c in range(KC):
                        nc.tensor.matmul(s_ps,
                                         lhsT=ct[:, c, s * 128:(s + 1) * 128],
                                         rhs=qa_t[:, c, :], start=(c == 0),
                                         stop=False)
                    nc.tensor.matmul(s_ps, lhsT=ones_row, rhs=qrow,
                                     start=False, stop=True)
                    Sm = spool.tile([128, M], F32, tag="Sm")
                    nc.vector.tensor_add(out=Sm, in0=s_ps[:, 0:M], in1=qmB)
                    nshmax = spool.tile([128, 1], F32, tag="nshmax")
                    nc.vector.reduce_max(out=nshmax, in_=Sm,
                                         axis=mybir.AxisListType.X, negate=True)
                    biasT = spool.tile([128, 1], F32, tag="biasT")
                    nc.vector.tensor_scalar_mul(out=biasT, in0=nshmax,
                                                scalar1=s0c[:, s:s + 1])
                    E = epool.tile([128, M], F32, tag="E")
                    Zrow = spool.tile([128, 1], F32, tag="Zrow")
                    nc.scalar.activation(out=E, in_=Sm, func=ACTF.Exp,
                                         bias=biasT, scale=s0c[:, s:s + 1],
                                         accum_out=Zrow)
                    Es[s], Zs[s] = E, Zrow
                    nc.vector.reduce_max(out=RM[:, s:s + 1], in_=s_ps[:, 0:M],
                                         axis=mybir.AxisListType.X)
                    nc.vector.tensor_copy(out=CL[:, s:s + 1],
                                          in_=s_ps[:, M:M + 1])
                t = step - 1
                if t < 0:
                    continue
                zr = spool.tile([128, 1], F32, tag="zr")
                nc.vector.reciprocal(out=zr, in_=Zs[t])
                et_ps = ps_t.tile([128, M], F32, tag="pst")
                nc.tensor.transpose(et_ps, Es[t], ident)
                et = etpool.tile([128, M], BF16, tag="et")
                nc.vector.tensor_copy(out=et, in_=et_ps)
                a_ps = ps_a.tile([128, D], F32, tag="aps")
                nc.tensor.matmul(a_ps[:, 0:512], lhsT=et, rhs=qb_t[:, 0:512],
                                 start=True, stop=True)
                nc.tensor.matmul(a_ps[:, 512:D], lhsT=et, rhs=qb_t[:, 512:D],
                                 start=True, stop=True)
                stage = stA.tile([128, 2 * D], F32, tag="stA")
                nc.scalar.activation(out=stage[:, 0:D], in_=a_ps,
                                     func=ACTF.Copy, scale=zr)
                nc.gpsimd.tensor_mul(out=stage[:, D:2 * D],
                                     in0=stage[:, 0:D], in1=c_big[:, t, :])
                nc.sync.dma_start(
                    out=out_d[b, t * 128:(t + 1) * 128, D:3 * D], in_=stage)

            # q2c logits, batch-wide: G = (RM + CL)*s0 - 1e9*cm
            G0 = spool.tile([128, NT], F32, tag="G0")
            nc.vector.tensor_add(out=G0, in0=RM, in1=CL)
            G1 = spool.tile([128, NT], F32, tag="G1")
            nc.vector.tensor_mul(out=G1, in0=G0, in1=s0c)
            G = spool.tile([128, NT], F32, tag="G")
            nc.vector.tensor_add(out=G, in0=G1, in1=cmNc)

            # ---------------- Q2C global softmax + Bctx ----------------
            # G is already in column space [128, NT]; global max via one tiny
            # transpose, then exp/sum stay in column space (no row-space trip).
            nmax = spool.tile([128, 1], F32, tag="nmax")
            nc.vector.reduce_max(out=nmax, in_=G, axis=mybir.AxisListType.X,
                                 negate=True)
            t1_ps = ps_t.tile([1, 128], F32, tag="pst")
            nc.tensor.transpose(t1_ps, nmax, ident)
            negMg = spool.tile([1, 1], F32, tag="negMg")
            nc.vector.tensor_reduce(out=negMg, in_=t1_ps,
                                    axis=mybir.AxisListType.X, op=AX.min)
            nmb_ps = ps_t.tile([128, 1], F32, tag="pst")
            nc.tensor.matmul(nmb_ps, lhsT=ones_row, rhs=negMg, start=True,
                             stop=True)
            nmb = spool.tile([128, 1], F32, tag="nmb")
            nc.vector.tensor_copy(out=nmb, in_=nmb_ps)
            er = spool.tile([128, NT], F32, tag="er")
            zq = spool.tile([128, 1], F32, tag="zq")
            nc.scalar.activation(out=er, in_=G, func=ACTF.Exp, bias=nmb,
                                 accum_out=zq)
            zs_ps = ps_t.tile([1, 1], F32, tag="pst")
            nc.tensor.matmul(zs_ps, lhsT=zq, rhs=ones_col, start=True,
                             stop=True)
            zqr = spool.tile([1, 1], F32, tag="zqr")
            nc.vector.reciprocal(out=zqr, in_=zs_ps)
            # Bctx = (er/Z) @ C in bf16 (1 cycle/row)
            er_bf = spool.tile([128, NT], BF16, tag="erbf")
            nc.vector.tensor_copy(out=er_bf, in_=er)
            bctx_ps = ps_a.tile([1, D], F32, tag="aps")
            for t in range(NT):
                nc.tensor.matmul(bctx_ps[:, 0:512],
                                 lhsT=er_bf[:, t:t + 1],
                                 rhs=cb[:, t, 0:512],
                                 start=(t == 0), stop=(t == NT - 1))
                nc.tensor.matmul(bctx_ps[:, 512:D],
                                 lhsT=er_bf[:, t:t + 1],
                                 rhs=cb[:, t, 512:D],
                                 start=(t == 0), stop=(t == NT - 1))
            bctx_bf = spool.tile([1, D], BF16, tag="bctxb")
            nc.scalar.activation(out=bctx_bf, in_=bctx_ps, func=ACTF.Copy,
                                 scale=zqr)
            # broadcast bctx row to 128 partitions (PE), then multiply per tile
            bb_ps = ps_a.tile([128, D], F32, tag="aps")
            nc.tensor.matmul(bb_ps[:, 0:512], lhsT=ones_row_bf,
                             rhs=bctx_bf[:, 0:512], start=True, stop=True)
            nc.tensor.matmul(bb_ps[:, 512:D], lhsT=ones_row_bf,
                             rhs=bctx_bf[:, 512:D], start=True, stop=True)
            Bb = qpool.tile([128, D], F32, tag="Bb")
            nc.scalar.copy(out=Bb, in_=bb_ps)
            for t in range(NT):
                sb = stB.tile([128, D], F32, tag="stB")
                cb_eng = nc.gpsimd if t % 2 == 1 else nc.vector
                cb_eng.tensor_mul(out=sb, in0=c_big[:, t, :], in1=Bb)
                # alternate C*B stores across the two HWDGE rings
                st_eng = nc.scalar if t % 2 == 0 else nc.sync
                st_eng.dma_start(
                    out=out_d[b, t * 128:(t + 1) * 128, 3 * D:4 * D], in_=sb)
    nc.compile()
    return nc


def _get_program() -> bass.Bass:
    if "nc" not in _CACHE:
        _CACHE["nc"] = _build_program()
    return _CACHE["nc"]


def _make_in_maps(inputs) -> list:
    C = np.ascontiguousarray(np.asarray(inputs["C"], dtype=np.float32))
    Q = np.ascontiguousarray(np.asarray(inputs["Q"], dtype=np.float32))
    c_mask = np.asarray(inputs["c_mask"])
    q_mask = np.asarray(inputs["q_mask"])
    w1 = np.asarray(inputs["w1"], dtype=np.float32).reshape(-1)
    w2 = np.asarray(inputs["w2"], dtype=np.float32).reshape(-1)
    w3 = np.asarray(inputs["w3"], dtype=np.float32).reshape(-1)
    B = C.shape[0]

    # Ct[b, c, p, n] = C[b, n, c*128+p] (pre-transposed score operand)
    Ct = np.ascontiguousarray(
        C.transpose(0, 2, 1).reshape(B, KC, 128, N))

    # qa[b, p, c, j] = Q[b, j, c*128+p] * w3[c*128+p]; col M = w1[c*128+p]
    qa = np.empty((B, 128, KC, M + 1), np.float32)
    Qt = Q.transpose(0, 2, 1).reshape(B, KC, 128, M)          # [B, c, p, j]
    qa[:, :, :, 0:M] = (Qt * w3.reshape(1, KC, 128, 1)).transpose(0, 2, 1, 3)
    qa[:, :, :, M] = w1.reshape(KC, 128).T[None, :, :]

    qrow = np.zeros((B, 1, M + 1), np.float32)
    qrow[:, 0, 0:M] = Q @ w2
    # q-mask pre-broadcast to all 128 partitions: device adds it with one
    # DVE op instead of a rank-1 matmul back into the score psum
    qmB = np.ascontiguousarray(np.broadcast_to(
        q_mask[:, 0:1, :].astype(np.float32) * NEG, (B, 128, M)))
    Qb = Q.astype(ml_dtypes.bfloat16)
    # cmT[b, p, t] = c_mask[b, 0, t*128 + p]
    cmT = np.ascontiguousarray(
        c_mask[:, 0, :].astype(np.float32).reshape(B, NT, 128).transpose(0, 2, 1))

    in_maps = []
    for core in range(NCORES):
        sl = slice(core * NB, (core + 1) * NB)
        in_maps.append({
            "C": C[sl],
            "Ct": np.ascontiguousarray(Ct[sl]),
            "qa": np.ascontiguousarray(qa[sl]),
            "qrow": np.ascontiguousarray(qrow[sl]),
            "qmB": np.ascontiguousarray(qmB[sl]),
            "Qb": np.ascontiguousarray(Qb[sl]),
            "cmT": np.ascontiguousarray(cmT[sl]),
        })
    return in_maps


def kernel(**inputs) -> np.ndarray:
    nc = _get_program()
    in_maps = _make_in_maps(inputs)
    res = run_bass_kernel_spmd(nc, in_maps, list(range(NCORES)))
    out = np.concatenate([r["out"] for r in res.results], axis=0)
    # C passthrough block, filled during the gather (bit-exact input copy)
    out[:, :, 0:D] = np.asarray(inputs["C"], dtype=np.float32)
    return out


# revision 50
# speedup vs baseline: 1.1143x; 1.0029x over previous
"""Trainium2 Bass kernel for nn_AttentionFlow (trilinear attention flow layer).

Full inputs -> shard batch over 8 NeuronCores (2 batches/core) -> gather.

Per batch (C [1024,768], Q [128,768]):
  S[i,j] = w1.C_i + w2.Q_j + (C_i*w3).Q_j   (c_logit dropped from columns:
           softmax over j is invariant to per-row constants)
  C2Q = softmax_j(masked S); A = C2Q @ Q
  Q2C = softmax_i(c-masked rowmax of raw S); Bctx = Q2C @ C
  out = [C | A | C*A | C*Bctx]

v2 layout strategy (vs the all-fp32 v1):
  - The Q-side prep (Q^T*w3 with the appended w1 column, q_logit row, q-mask
    row) is precomputed on the host; nothing q-side is transposed on device.
  - Scores stay fp32 on the PE: the Q2C path takes exp of per-row maxima, so
    ~0.1-abs bf16 score noise turns into >5% weight flips between competing
    context rows and blows the C*B tolerance (measured: 1.3 abs err).
  - The A matmul runs bf16 (E^T copied to bf16, host-cast bf16 Q): softmax
    weights tolerate 0.4% relative error fine (measured 4e-3 end-to-end).
  - Bctx runs bf16 (f32r needs explicitly pre-rounded producers per the BIR
    verifier, which costs the same extra copy); C tiles are converted to bf16
    on the ACT engine during the transpose lookahead stage.
  - Output blocks stream independently: [A | C*A] from one staging tile,
    C*B later; all on the SP HWDGE ring while loads use the ACT ring.
  - The C passthrough block (out[:, :, 0:D] = C) is a bit-exact copy of an
    input, so it is filled during the host-side gather/concat step instead of
    being round-tripped through the device store path.
"""

from contextlib import ExitStack

import numpy as np
import ml_dtypes

import concourse.bass as bass
import concourse.tile as tile
from concourse import bacc, mybir
from concourse.bass_utils import run_bass_kernel_spmd
from concourse.masks import make_identity

F32 = mybir.dt.float32
BF16 = mybir.dt.bfloat16
AX = mybir.AluOpType
ACTF = mybir.ActivationFunctionType

NEG = np.float32(-1e9)
NCORES = 8
NB = 2           # batches per core
N = 1024         # context length
M = 128          # query length
D = 768          # feature dim
NT = N // 128    # n-tiles per batch
KC = D // 128    # contraction chunks

_CACHE: dict = {}


def _build_program(iters: int = 1) -> bass.Bass:
    nc = bacc.Bacc("TRN2", target_bir_lowering=False, debug=False)
    C_d = nc.declare_dram_parameter("C", [NB, N, D], F32, isOutput=False)
    Ct_d = nc.declare_dram_parameter("Ct", [NB, KC, 128, N], F32,
                                     isOutput=False)
    qa_d = nc.declare_dram_parameter("qa", [NB, 128, KC, M + 1], F32,
                                     isOutput=False)
    qrow_d = nc.declare_dram_parameter("qrow", [NB, 1, M + 1], F32,
                                       isOutput=False)
    qmB_d = nc.declare_dram_parameter("qmB", [NB, 128, M], F32,
                                      isOutput=False)
    Qb_d = nc.declare_dram_parameter("Qb", [NB, M, D], BF16, isOutput=False)
    cmT_d = nc.declare_dram_parameter("cmT", [NB, 128, NT], F32, isOutput=False)
    out_d = nc.declare_dram_parameter("out", [NB, N, 4 * D], F32, isOutput=True)

    with ExitStack() as ctx:
        tc = ctx.enter_context(tile.TileContext(nc))
        consts = ctx.enter_context(tc.tile_pool(name="consts", bufs=1))
        cpool = ctx.enter_context(tc.tile_pool(name="cpool", bufs=2))
        cbpool = ctx.enter_context(tc.tile_pool(name="cbpool", bufs=2))
        ctpool = ctx.enter_context(tc.tile_pool(name="ctpool", bufs=2))
        qpool = ctx.enter_context(tc.tile_pool(name="qpool", bufs=2))
        epool = ctx.enter_context(tc.tile_pool(name="epool", bufs=6))
        etpool = ctx.enter_context(tc.tile_pool(name="etpool", bufs=6))
        spool = ctx.enter_context(tc.tile_pool(name="spool", bufs=4))
        stA = ctx.enter_context(tc.tile_pool(name="stA", bufs=4))
        stB = ctx.enter_context(tc.tile_pool(name="stB", bufs=3))
        ps_t = ctx.enter_context(tc.tile_pool(name="ps_t", bufs=1, space="PSUM"))
        ps_s = ctx.enter_context(tc.tile_pool(name="ps_s", bufs=3, space="PSUM"))
        ps_a = ctx.enter_context(tc.tile_pool(name="ps_a", bufs=2, space="PSUM"))

        ident = consts.tile([128, 128], F32)
        make_identity(nc, ident)
        ones_row = consts.tile([1, 128], F32)
        nc.vector.memset(ones_row, 1.0)
        ones_col = consts.tile([128, 1], F32)
        nc.vector.memset(ones_col, 1.0)
        ones_row_bf = consts.tile([1, 128], BF16)
        nc.vector.memset(ones_row_bf, 1.0)

        loop_ctx = tc.For_i(0, iters, 1) if iters > 1 else None
        if loop_ctx is not None:
            ctx.enter_context(loop_ctx)
        for b in range(NB):
            # ------- loads (ACT HWDGE ring; stores use SP HWDGE ring) -------
            c_big = cpool.tile([128, NT, D], F32, tag="c")
            for t in range(NT):
                nc.scalar.dma_start(out=c_big[:, t, :],
                                    in_=C_d[b, t * 128:(t + 1) * 128, :])
            # host-pre-transposed C^T: kills 48 PE transposes + psum copies
            ct = ctpool.tile([128, KC, N], F32, tag="ct")
            nc.scalar.dma_start(
                out=ct, in_=Ct_d[b].rearrange("c p n -> p c n"))
            qa_t = qpool.tile([128, KC, M + 1], F32, tag="qa")
            nc.scalar.dma_start(out=qa_t, in_=qa_d[b])
            qb_t = qpool.tile([M, D], BF16, tag="qb")
            nc.scalar.dma_start(out=qb_t, in_=Qb_d[b])
            qrow = qpool.tile([1, M + 1], F32, tag="qrow")
            nc.scalar.dma_start(out=qrow, in_=qrow_d[b])
            qmB = qpool.tile([128, M], F32, tag="qmB")
            nc.scalar.dma_start(out=qmB, in_=qmB_d[b])
            cmT = spool.tile([128, NT], F32, tag="cmT")
            nc.scalar.dma_start(out=cmT, in_=cmT_d[b])

            # mask derivations: s0=1-cm, cmN=-1e9*cm
            s0c = spool.tile([128, NT], F32, tag="s0c")
            nc.vector.tensor_scalar(out=s0c, in0=cmT, scalar1=-1.0, scalar2=1.0,
                                    op0=AX.mult, op1=AX.add)
            cmNc = spool.tile([128, NT], F32, tag="cmNc")
            nc.vector.tensor_scalar_mul(out=cmNc, in0=cmT, scalar1=float(NEG))

            cb = cbpool.tile([128, NT, D], BF16, tag="cb")
            # per-tile row maxima / c_logits accumulate into columns; the G
            # combine runs once per batch (3 wide DVE ops vs 16 tiny ones)
            RM = spool.tile([128, NT], F32, tag="RM")
            CL = spool.tile([128, NT], F32, tag="CL")
            # software-pipelined: the E^T/A stage for tile t issues AFTER the
            # score matmuls of tile t+1, so the in-order PE stream never waits
            # on the exp chain; DVE issues critical ops (exp path) first
            Es, Zs = [None] * NT, [None] * NT
            for step in range(NT + 1):
                s = step
                if s < NT:
                    nc.gpsimd.tensor_copy(out=cb[:, s, :], in_=c_big[:, s, :])
                    s_ps = ps_s.tile([128, M + 1], F32, tag="sps")
                    for c in range(KC):
                        nc.tensor.matmul(s_ps,
                                         lhsT=ct[:, c, s * 128:(s + 1) * 128],
                                         rhs=qa_t[:, c, :], start=(c == 0),
                                         stop=False)
                    nc.tensor.matmul(s_ps, lhsT=ones_row, rhs=qrow,
                                     start=False, stop=True)
                    Sm = spool.tile([128, M], F32, tag="Sm")
                    nc.vector.tensor_add(out=Sm, in0=s_ps[:, 0:M], in1=qmB)
                    nshmax = spool.tile([128, 1], F32, tag="nshmax")
                    nc.vector.reduce_max(out=nshmax, in_=Sm,
                                         axis=mybir.AxisListType.X, negate=True)
                    biasT = spool.tile([128, 1], F32, tag="biasT")
                    nc.vector.tensor_scalar_mul(out=biasT, in0=nshmax,
                                                scalar1=s0c[:, s:s + 1])
                    E = epool.tile([128, M], F32, tag="E")
                    Zrow = spool.tile([128, 1], F32, tag="Zrow")
                    nc.scalar.activation(out=E, in_=Sm, func=ACTF.Exp,
                                         bias=biasT, scale=s0c[:, s:s + 1],
                                         accum_out=Zrow)
                    Es[s], Zs[s] = E, Zrow
                    nc.vector.reduce_max(out=RM[:, s:s + 1], in_=s_ps[:, 0:M],
                                         axis=mybir.AxisListType.X)
                    nc.vector.tensor_copy(out=CL[:, s:s + 1],
                                          in_=s_ps[:, M:M + 1])
                t = step - 1
                if t < 0:
                    continue
                zr = spool.tile([128, 1], F32, tag="zr")
                nc.vector.reciprocal(out=zr, in_=Zs[t])
                et_ps = ps_t.tile([128, M], F32, tag="pst")
                nc.tensor.transpose(et_ps, Es[t], ident)
                et = etpool.tile([128, M], BF16, tag="et")
                nc.vector.tensor_copy(out=et, in_=et_ps)
                a_ps = ps_a.tile([128, D], F32, tag="aps")
                nc.tensor.matmul(a_ps[:, 0:512], lhsT=et, rhs=qb_t[:, 0:512],
                                 start=True, stop=True)
                nc.tensor.matmul(a_ps[:, 512:D], lhsT=et, rhs=qb_t[:, 512:D],
                                 start=True, stop=True)
                stage = stA.tile([128, 2 * D], F32, tag="stA")
                nc.scalar.activation(out=stage[:, 0:D], in_=a_ps,
                                     func=ACTF.Copy, scale=zr)
                nc.gpsimd.tensor_mul(out=stage[:, D:2 * D],
                                     in0=stage[:, 0:D], in1=c_big[:, t, :])
                nc.sync.dma_start(
                    out=out_d[b, t * 128:(t + 1) * 128, D:3 * D], in_=stage)

            # q2c logits, batch-wide: G = (RM + CL)*s0 - 1e9*cm
            G0 = spool.tile([128, NT], F32, tag="G0")
            nc.vector.tensor_add(out=G0, in0=RM, in1=CL)
            G1 = spool.tile([128, NT], F32, tag="G1")
            nc.vector.tensor_mul(out=G1, in0=G0, in1=s0c)
            G = spool.tile([128, NT], F32, tag="G")
            nc.vector.tensor_add(out=G, in0=G1, in1=cmNc)

            # ---------------- Q2C global softmax + Bctx ----------------
            # G is already in column space [128, NT]; global max via one tiny
            # transpose, then exp/sum stay in column space (no row-space trip).
            nmax = spool.tile([128, 1], F32, tag="nmax")
            nc.vector.reduce_max(out=nmax, in_=G, axis=mybir.AxisListType.X,
                                 negate=True)
            t1_ps = ps_t.tile([1, 128], F32, tag="pst")
            nc.tensor.transpose(t1_ps, nmax, ident)
            negMg = spool.tile([1, 1], F32, tag="negMg")
            nc.vector.tensor_reduce(out=negMg, in_=t1_ps,
                                    axis=mybir.AxisListType.X, op=AX.min)
            nmb_ps = ps_t.tile([128, 1], F32, tag="pst")
            nc.tensor.matmul(nmb_ps, lhsT=ones_row, rhs=negMg, start=True,
                             stop=True)
            nmb = spool.tile([128, 1], F32, tag="nmb")
            nc.vector.tensor_copy(out=nmb, in_=nmb_ps)
            er = spool.tile([128, NT], F32, tag="er")
            zq = spool.tile([128, 1], F32, tag="zq")
            nc.scalar.activation(out=er, in_=G, func=ACTF.Exp, bias=nmb,
                                 accum_out=zq)
            zs_ps = ps_t.tile([1, 1], F32, tag="pst")
            nc.tensor.matmul(zs_ps, lhsT=zq, rhs=ones_col, start=True,
                             stop=True)
            zqr = spool.tile([1, 1], F32, tag="zqr")
            nc.vector.reciprocal(out=zqr, in_=zs_ps)
            # Bctx = (er/Z) @ C in bf16 (1 cycle/row)
            er_bf = spool.tile([128, NT], BF16, tag="erbf")
            nc.vector.tensor_copy(out=er_bf, in_=er)
            bctx_ps = ps_a.tile([1, D], F32, tag="aps")
            for t in range(NT):
                nc.tensor.matmul(bctx_ps[:, 0:512],
                                 lhsT=er_bf[:, t:t + 1],
                                 rhs=cb[:, t, 0:512],
                                 start=(t == 0), stop=(t == NT - 1))
                nc.tensor.matmul(bctx_ps[:, 512:D],
                                 lhsT=er_bf[:, t:t + 1],
                                 rhs=cb[:, t, 512:D],
                                 start=(t == 0), stop=(t == NT - 1))
            bctx_bf = spool.tile([1, D], BF16, tag="bctxb")
            nc.scalar.activation(out=bctx_bf, in_=bctx_ps, func=ACTF.Copy,
                                 scale=zqr)
            # broadcast bctx row to 128 partitions (PE), then multiply per tile
            bb_ps = ps_a.tile([128, D], F32, tag="aps")
            nc.tensor.matmul(bb_ps[:, 0:512], lhsT=ones_row_bf,
                             rhs=bctx_bf[:, 0:512], start=True, stop=True)
            nc.tensor.matmul(bb_ps[:, 512:D], lhsT=ones_row_bf,
                             rhs=bctx_bf[:, 512:D], start=True, stop=True)
            Bb = qpool.tile([128, D], F32, tag="Bb")
            nc.scalar.copy(out=Bb, in_=bb_ps)
            for t in range(NT):
                sb = stB.tile([128, D], F32, tag="stB")
                cb_eng = nc.gpsimd if t % 2 == 1 else nc.vector
                cb_eng.tensor_mul(out=sb, in0=cb[:, t, :], in1=Bb)
                # alternate C*B stores across the two HWDGE rings
                st_eng = nc.scalar if t % 2 == 0 else nc.sync
                st_eng.dma_start(
                    out=out_d[b, t * 128:(t + 1) * 128, 3 * D:4 * D], in_=sb)
    nc.compile()
    return nc


def _get_program() -> bass.Bass:
    if "nc" not in _CACHE:
        _CACHE["nc"] = _build_program()
    return _CACHE["nc"]


def _make_in_maps(inputs) -> list:
    C = np.ascontiguousarray(np.asarray(inputs["C"], dtype=np.float32))
    Q = np.ascontiguousarray(np.asarray(inputs["Q"], dtype=np.float32))
    c_mask = np.asarray(inputs["c_mask"])
    q_mask = np.asarray(inputs["q_mask"])
    w1 = np.asarray(inputs["w1"], dtype=np.float32).reshape(-1)
    w2 = np.asarray(inputs["w2"], dtype=np.float32).reshape(-1)
    w3 = np.asarray(inputs["w3"], dtype=np.float32).reshape(-1)
    B = C.shape[0]

    # Ct[b, c, p, n] = C[b, n, c*128+p] (pre-transposed score operand)
    Ct = np.ascontiguousarray(
        C.transpose(0, 2, 1).reshape(B, KC, 128, N))

    # qa[b, p, c, j] = Q[b, j, c*128+p] * w3[c*128+p]; col M = w1[c*128+p]
    qa = np.empty((B, 128, KC, M + 1), np.float32)
    Qt = Q.transpose(0, 2, 1).reshape(B, KC, 128, M)          # [B, c, p, j]
    qa[:, :, :, 0:M] = (Qt * w3.reshape(1, KC, 128, 1)).transpose(0, 2, 1, 3)
    qa[:, :, :, M] = w1.reshape(KC, 128).T[None, :, :]

    qrow = np.zeros((B, 1, M + 1), np.float32)
    qrow[:, 0, 0:M] = Q @ w2
    # q-mask pre-broadcast to all 128 partitions: device adds it with one
    # DVE op instead of a rank-1 matmul back into the score psum
    qmB = np.ascontiguousarray(np.broadcast_to(
        q_mask[:, 0:1, :].astype(np.float32) * NEG, (B, 128, M)))
    Qb = Q.astype(ml_dtypes.bfloat16)
    # cmT[b, p, t] = c_mask[b, 0, t*128 + p]
    cmT = np.ascontiguousarray(
        c_mask[:, 0, :].astype(np.float32).reshape(B, NT, 128).transpose(0, 2, 1))

    in_maps = []
    for core in range(NCORES):
        sl = slice(core * NB, (core + 1) * NB)
        in_maps.append({
            "C": C[sl],
            "Ct": np.ascontiguousarray(Ct[sl]),
            "qa": np.ascontiguousarray(qa[sl]),
            "qrow": np.ascontiguousarray(qrow[sl]),
            "qmB": np.ascontiguousarray(qmB[sl]),
            "Qb": np.ascontiguousarray(Qb[sl]),
            "cmT": np.ascontiguousarray(cmT[sl]),
        })
    return in_maps


def kernel(**inputs) -> np.ndarray:
    nc = _get_program()
    in_maps = _make_in_maps(inputs)
    res = run_bass_kernel_spmd(nc, in_maps, list(range(NCORES)))
    out = np.concatenate([r["out"] for r in res.results], axis=0)
    # C passthrough block, filled during the gather (bit-exact input copy)
    out[:, :, 0:D] = np.asarray(inputs["C"], dtype=np.float32)
    return out
